# revision 34
# baseline (speedup 1.0000x reference)
"""Fused transformer block (attention + FFN + 2x LayerNorm) on 8 Trainium2
NeuronCores via Bass/Tile.

Sharding: 8 cores = (batch b in 0..3) x (query-half s in 0..1).  Each core
receives the full x[b] (needed for K/V), computes outputs for its half of the
2048 tokens, fully fused on-chip (no collectives).  Matmuls run in bf16 with
fp32 PSUM accumulation; softmax/LayerNorm/residual math in fp32.

Attention layout trick: scores are computed transposed (S^T = K^T.T @ Q^T per
head, keys on partitions), softmax uses exp without max-subtraction (scores
are O(1) by construction), the normalizer is obtained by appending a ones
column to V (row 64 of the AV product = sum of exp), and the AV product comes
out as O^T [head_dim, tokens] which is exactly the lhsT layout the output
projection wants - so no transposes anywhere in attention.

v1.5 changes vs the original baseline:
 - per-head normalizer: ACT row-copy + Pool partition_broadcast + DVE
   reciprocal on [64, n] (the old [1, TQ] single-partition reciprocal was
   3.3us and serialized heads); AV accumulates per query-half in 1-bank
   psum tiles shared with the kq-production pool.
 - kq production runs one chunk ahead of head consumption (tags rotate %3)
   so the PE never idles across the head-transition normalize chains.
 - weight DMAs spread across queues and issued early: FF1 weights prefetch
   during attention/out-proj; FF2 weights half-prefetch during FF1 and the
   kernel streams each FF2 weight byte exactly once (C-split FF2).
 - b_out folded into the residual input host-side; no y-prefill trick.
"""

import sys

for _p in ("/opt/trn_rl_repo",):
    if _p not in sys.path:
        sys.path.insert(0, _p)

import numpy as np
import ml_dtypes

import concourse.bass as bass
import concourse.mybir as mybir
import concourse.tile as tile
from concourse import bacc
from concourse.bass_utils import run_bass_kernel_spmd
from concourse.masks import make_identity

FP32 = mybir.dt.float32
BF16 = mybir.dt.bfloat16
AF = mybir.ActivationFunctionType
OP = mybir.AluOpType

P = 128
NMAX = 512  # max matmul free dim / psum bank fp32 words
LN_EPS = 1e-5


def _chunks(total, size):
    out = []
    o = 0
    while o < total:
        s = min(size, total - o)
        out.append((o, s))
        o += s
    return out


def build_nc(T, TQ, C, H, F, n_cores=8, reps=1, exp_as_copy=False):
    """Build the SPMD single-core program.  D (head dim) = C // H must be 64.

    reps > 1 emits the whole computation multiple times back-to-back inside
    one NEFF (same inputs/outputs) - used only for wall-clock timing."""
    D = C // H
    assert D == 64 and C % P == 0 and T % P == 0 and TQ % P == 0 and F % P == 0
    KC = C // P     # contraction chunks over C
    TB = T // P     # key-token blocks
    TQB = TQ // P   # query-token blocks
    MF = F // P     # FFN hidden blocks
    HPC = P // D    # heads per 128-chunk (=2)

    nc = bacc.Bacc("TRN2", target_bir_lowering=False, debug=False,
                   num_devices=n_cores)

    # ---- DRAM I/O ----
    xTp = nc.dram_tensor("xTp", [C, T], BF16, kind="ExternalInput")
    xres = nc.dram_tensor("xres", [TQ, C], FP32, kind="ExternalInput")
    wqkv = nc.dram_tensor("wqkv", [C, 3 * C], BF16, kind="ExternalInput")
    wout = nc.dram_tensor("wout", [C, C], BF16, kind="ExternalInput")
    wff1 = nc.dram_tensor("wff1", [C, F], BF16, kind="ExternalInput")
    wff2 = nc.dram_tensor("wff2", [F, C], BF16, kind="ExternalInput")
    bqkv = nc.dram_tensor("bqkv", [3 * C], FP32, kind="ExternalInput")
    bff1 = nc.dram_tensor("bff1", [F], FP32, kind="ExternalInput")
    g1 = nc.dram_tensor("g1", [C], FP32, kind="ExternalInput")
    bff2 = nc.dram_tensor("bff2", [C], FP32, kind="ExternalInput")
    g2 = nc.dram_tensor("g2", [C], FP32, kind="ExternalInput")
    be2 = nc.dram_tensor("be2", [C], FP32, kind="ExternalInput")
    y = nc.dram_tensor("y", [TQ, C], FP32, kind="ExternalOutput")

    def col_view(t, n, off=0):
        # [n*P] dram vector -> [P, n] view: (p, m) = t[off + m*P + p]
        return bass.AP(tensor=t[:].tensor, offset=off, ap=[[1, P], [P, n]])

    def bcast_view(t, n):
        # [n] dram vector broadcast across partitions -> [P, n]
        return bass.AP(tensor=t[:].tensor, offset=0, ap=[[0, P], [1, n]])

    import contextlib

    def emit_body(tc):
        with contextlib.ExitStack() as top:
            params = top.enter_context(tc.tile_pool(name="params", bufs=1))

            bq_sb = params.tile([P, KC], FP32, name="bq_sb", tag="bq_sb")
            bk_sb = params.tile([P, KC], FP32, name="bk_sb", tag="bk_sb")
            bv_sb = params.tile([P, KC], FP32, name="bv_sb", tag="bv_sb")
            bff1_sb = params.tile([P, MF], FP32, name="bff1_sb",
                                  tag="bff1_sb")
            eps_sb = params.tile([P, 1], FP32, name="eps_sb", tag="eps_sb")
            nc.vector.memset(eps_sb[:], LN_EPS)
            ident = params.tile([P, P], BF16, name="ident", tag="ident")
            make_identity(nc, ident[:])

            def layernorm(dst, src, g_bc, stats_pool):
                """dst[P, C] (any dtype) = LN(src[P, C] fp32) * g."""
                nsub = (C + NMAX - 1) // NMAX
                stats = stats_pool.tile([P, nsub, 6], FP32, name="ln_stats",
                                        tag="ln_stats", bufs=3)
                for i, (o, sz) in enumerate(_chunks(C, NMAX)):
                    nc.vector.bn_stats(out=stats[:, i, :],
                                       in_=src[:, o:o + sz])
                mv = stats_pool.tile([P, 2], FP32, name="ln_mv", tag="ln_mv",
                                     bufs=3)
                nc.vector.bn_aggr(out=mv[:], in_=stats[:])
                rstd = stats_pool.tile([P, 1], FP32, name="ln_rstd",
                                       tag="ln_rstd", bufs=3)
                nc.scalar.activation(out=rstd[:], in_=mv[:, 1:2],
                                     func=AF.Sqrt, bias=eps_sb[:], scale=1.0)
                nc.vector.reciprocal(out=rstd[:], in_=rstd[:])
                nc.vector.tensor_scalar(out=dst[:], in0=src[:],
                                        scalar1=mv[:, 0:1],
                                        scalar2=rstd[:],
                                        op0=OP.subtract, op1=OP.mult)
                if g_bc is not None:
                    nc.vector.scalar_tensor_tensor(
                        out=dst[:], in0=dst[:], scalar=0.0, in1=g_bc[:],
                        op0=OP.add, op1=OP.mult)

            # Right-side SBUF stack, bottom-up: w1gA (lives to FF1 end),
            # hT (ph3..FF1), attnT+wout (attention..ph3), later w1gB.
            # Stack discipline: each closes before anything below it.
            w1gA_scope = contextlib.ExitStack()
            w1g_poolA = w1gA_scope.enter_context(
                tc.tile_pool(name="w1gA", bufs=1, side="right"))
            hT_scope = contextlib.ExitStack()
            hT_pool = hT_scope.enter_context(
                tc.tile_pool(name="hTp", bufs=1, side="right"))
            hT_sb = [hT_pool.tile([P, TQ], BF16, name=f"hT{c}", tag=f"hT{c}")
                     for c in range(KC)]
            attn_scope = contextlib.ExitStack()
            attn_pool = attn_scope.enter_context(
                tc.tile_pool(name="attn", bufs=1, side="right"))
            attnT = [attn_pool.tile([P, TQ], BF16, name=f"attnT{m}",
                                    tag=f"attnT{m}") for m in range(KC)]
            wout_pool = attn_scope.enter_context(
                tc.tile_pool(name="woutp", bufs=1, side="right"))
            wout_sb = [wout_pool.tile([P, C], BF16, name=f"wout{kc}",
                                      tag=f"wout{kc}") for kc in range(KC)]

            # FF1 weight set A (2 of 4 groups): issued on the Pool queue
            # during attention, after the V-phase weights free.
            NG1 = 8  # f-chunks per w1g group
            n_groups = (MF + NG1 - 1) // NG1

            # ========== phases 1+2: QKV projections + attention ==========
            scale = 1.0 / float(np.sqrt(D))
            qkv_scope = contextlib.ExitStack()
            qkv_pool = qkv_scope.enter_context(
                tc.tile_pool(name="qkv", bufs=1))
            v_pack = [qkv_pool.tile([P, H, D + 1], BF16, name=f"v_pack{tb}",
                                    tag=f"v_pack{tb}") for tb in range(TB)]

            xT_pool = qkv_scope.enter_context(tc.tile_pool(name="xT",
                                                           bufs=1))
            w_pool = qkv_scope.enter_context(
                tc.tile_pool(name="wstream", bufs=1))
            # scores psum: [128, TQ] (2 banks) x2
            pss_pool = qkv_scope.enter_context(
                tc.tile_pool(name="pss", bufs=2, space="PSUM"))
            # shared 1-bank psum rotation: kq production rounds, V-phase
            # rounds, and per-half AV accumulators
            ps_pool = qkv_scope.enter_context(
                tc.tile_pool(name="ps1", bufs=4, space="PSUM"))
            es_pool = qkv_scope.enter_context(tc.tile_pool(name="expS",
                                                           bufs=3))
            nrm_pool = qkv_scope.enter_context(tc.tile_pool(name="nrm",
                                                            bufs=1))

            xT_sb = [xT_pool.tile([P, T], BF16, name=f"xT{kc}",
                                  tag=f"xT{kc}") for kc in range(KC)]
            XH = min(2 * P, T)
            for kc in range(KC):
                nc.sync.dma_start(out=xT_sb[kc][:, :XH],
                                  in_=xTp[kc * P:(kc + 1) * P, :XH])
            for kc in range(KC):
                nc.sync.dma_start(out=xT_sb[kc][:, XH:],
                                  in_=xTp[kc * P:(kc + 1) * P, XH:])

            # --- V (normal layout, packed per head with a ones column) ---
            wv_scope = contextlib.ExitStack()
            wv_pool = wv_scope.enter_context(tc.tile_pool(name="wv", bufs=1))
            wv = [wv_pool.tile([P, C], BF16, name=f"wv{kc}", tag=f"wv{kc}")
                  for kc in range(KC)]
            WH = min(NMAX, C)
            for kc in range(KC):
                nc.scalar.dma_start(
                    out=wv[kc][:, :WH],
                    in_=wqkv[kc * P:(kc + 1) * P, 2 * C:2 * C + WH])
            for kc in range(KC):
                if WH < C:
                    nc.scalar.dma_start(
                        out=wv[kc][:, WH:],
                        in_=wqkv[kc * P:(kc + 1) * P, 2 * C + WH:3 * C])
            nc.scalar.dma_start(out=bq_sb[:], in_=col_view(bqkv, KC, 0))
            nc.scalar.dma_start(out=bk_sb[:], in_=col_view(bqkv, KC, C))
            nc.scalar.dma_start(out=bv_sb[:], in_=col_view(bqkv, KC, 2 * C))
            nc.scalar.dma_start(out=bff1_sb[:], in_=col_view(bff1, MF, 0))
            # K/Q weights + wout + LN broadcasts on the SP queue (idle during
            # attention); K first (needed first).
            wk = [w_pool.tile([P, C], BF16, name=f"wk{kc}", tag=f"wk{kc}")
                  for kc in range(KC)]
            for kc in range(KC):
                nc.sync.dma_start(out=wk[kc][:],
                                  in_=wqkv[kc * P:(kc + 1) * P, C:2 * C])
            wq = [w_pool.tile([P, C], BF16, name=f"wq{kc}", tag=f"wq{kc}")
                  for kc in range(KC)]
            for kc in range(KC):
                nc.scalar.dma_start(out=wq[kc][:],
                                  in_=wqkv[kc * P:(kc + 1) * P, 0:C])
            for kc in range(KC):
                nc.sync.dma_start(out=wout_sb[kc][:],
                                  in_=wout[kc * P:(kc + 1) * P, :])

            for tb in range(TB):
                nc.vector.memset(v_pack[tb][:, :, D:D + 1], 1.0)
                for (no, nsz) in _chunks(C, NMAX):
                    psv = ps_pool.tile([P, NMAX], FP32, name="psv",
                                       tag="ps1", bufs=4)
                    for kc in range(KC):
                        nc.tensor.matmul(
                            psv[:, :nsz],
                            xT_sb[kc][:, tb * P:(tb + 1) * P],
                            wv[kc][:, no:no + nsz],
                            start=(kc == 0), stop=(kc == KC - 1))
                    hview = v_pack[tb][:, no // D:(no + nsz) // D, 0:D]
                    nc.vector.tensor_copy(
                        out=hview,
                        in_=psv[:, :nsz].rearrange("p (h d) -> p h d", d=D))
            wv_scope.close()

            # FF1 weight set A: fresh tiles, issued now on the Pool queue so
            # the transfers land during attention.  Groups 2/3 rotate into
            # the same tags later (WAR-gated on FF1's reads).
            w1g_sets = {}
            for si, setpool in (("A", w1g_poolA),):
                w1g_sets[si] = [
                    setpool.tile([P, NG1 * P], BF16, name=f"w1g{si}_{kc}",
                                 tag=f"w1g{si}{kc}", bufs=1)
                    for kc in range(KC)]
            for kc in range(KC):
                nc.gpsimd.dma_start(
                    out=w1g_sets["A"][kc][:],
                    in_=wff1[kc * P:(kc + 1) * P, 0:NG1 * P])

            kq_pool = qkv_scope.enter_context(tc.tile_pool(name="kq",
                                                           bufs=1))

            def kq_tiles(m):
                kT_m = kq_pool.tile([P, T], BF16, name=f"kT_{m}",
                                    tag=f"kT{m % 2}")
                qT_m = kq_pool.tile([P, TQ], BF16, name=f"qT_{m}",
                                    tag=f"qT{m % 2}")
                return kT_m, qT_m

            def emit_kq_half(m, kT_m, qT_m, half):
                """Produce half of kT_m/qT_m.  Split so the emission can be
                interleaved between attention heads - the PE then always has
                independent matmuls to chew on while a head's normalize
                chain drains."""
                kch = _chunks(T, NMAX)
                qch = _chunks(TQ, NMAX)
                kh = kch[:len(kch) // 2] if half == 0 else kch[len(kch) // 2:]
                qh = qch[:len(qch) // 2] if half == 0 else qch[len(qch) // 2:]
                for (no, nsz) in kh:
                    psk = ps_pool.tile([P, NMAX], FP32, name="psk",
                                       tag="ps1", bufs=4)
                    for kc in range(KC):
                        nc.tensor.matmul(
                            psk[:, :nsz],
                            wk[kc][:, m * P:(m + 1) * P],
                            xT_sb[kc][:, no:no + nsz],
                            start=(kc == 0), stop=(kc == KC - 1))
                    nc.vector.tensor_scalar(
                        out=kT_m[:, no:no + nsz], in0=psk[:, :nsz],
                        scalar1=bk_sb[:, m:m + 1], scalar2=None,
                        op0=OP.add)
                for (no, nsz) in qh:
                    psq = ps_pool.tile([P, NMAX], FP32, name="psq",
                                       tag="ps1", bufs=4)
                    for kc in range(KC):
                        nc.tensor.matmul(
                            psq[:, :nsz],
                            wq[kc][:, m * P:(m + 1) * P],
                            xT_sb[kc][:, no:no + nsz],
                            start=(kc == 0), stop=(kc == KC - 1))
                    nc.vector.tensor_scalar(
                        out=qT_m[:, no:no + nsz], in0=psq[:, :nsz],
                        scalar1=bq_sb[:, m:m + 1], scalar2=None,
                        op0=OP.add)


            def emit_att_head(h, kT_m, qT_m):
                """Scores + exp + AV for one head.  attnT gets the
                UN-normalized O^T; the 1/sum(exp) row is stashed in rrec.
                Normalization happens per-chunk later (emit_att_finalize),
                off the PE critical path - the pso psum slots free after a
                single ACT reciprocal + DVE copy."""
                m, hoff = h // HPC, (h % HPC) * D
                # per query-half AV accumulators (1 psum bank each, shared
                # rotation with kq production)
                pso = [ps_pool.tile([P, NMAX], FP32, name=f"pso{i}",
                                    tag="ps1", bufs=4)
                       for i in range(TQ // NMAX)]
                for ts in range(TB):
                    esr = es_pool.tile([P, TQ], BF16, name="esr",
                                       tag="esr", bufs=3)
                    pss = pss_pool.tile([P, TQ], FP32, name="pss",
                                        tag="pss", bufs=2)
                    for (no, nsz) in _chunks(TQ, NMAX):
                        nc.tensor.matmul(
                            pss[:, no:no + nsz],
                            kT_m[hoff:hoff + D, ts * P:(ts + 1) * P],
                            qT_m[hoff:hoff + D, no:no + nsz],
                            start=True, stop=True)
                    nc.scalar.activation(
                        out=esr[:], in_=pss[:],
                        func=(AF.Copy if exp_as_copy else AF.Exp),
                        scale=scale)
                    for i, (no, nsz) in enumerate(_chunks(TQ, NMAX)):
                        nc.tensor.matmul(
                            pso[i][:D + 1, :nsz],
                            v_pack[ts][:, h, :],
                            esr[:, no:no + nsz],
                            start=(ts == 0), stop=(ts == TB - 1))
                rbh = rb_tiles[h]
                for i, (no, nsz) in enumerate(_chunks(TQ, NMAX)):
                    rraw = nrm_pool.tile([1, NMAX], FP32, name="rraw",
                                         tag="rraw", bufs=2)
                    nc.vector.tensor_copy(out=rraw[:, :nsz],
                                          in_=pso[i][D:D + 1, :nsz])
                    rinv = nrm_pool.tile([1, NMAX], FP32, name="rinv",
                                         tag="rinv", bufs=2)
                    nc.vector.reciprocal_approx_fast(out=rinv[:, :nsz],
                                                     in_=rraw[:, :nsz])
                    rbf = nrm_pool.tile([1, NMAX], BF16, name="rbf",
                                        tag="rbf", bufs=2)
                    nc.scalar.copy(out=rbf[:, :nsz], in_=rinv[:, :nsz])
                    nc.gpsimd.partition_broadcast(
                        rbh[:, no:no + nsz], rbf[:, :nsz], channels=P)
                    nc.vector.tensor_copy(
                        out=attnT[m][hoff:hoff + D, no:no + nsz],
                        in_=pso[i][0:D, :nsz])

            def emit_att_finalize(m):
                """Normalize chunk m of attnT (heads 2m, 2m+1) and add the
                V bias.  Runs on DVE, overlapped with later heads."""
                for hh in range(HPC):
                    hoff = hh * D
                    nc.vector.tensor_tensor(
                        out=attnT[m][hoff:hoff + D, :],
                        in0=attnT[m][hoff:hoff + D, :],
                        in1=rb_tiles[m * HPC + hh][hoff:hoff + D, :],
                        op=OP.mult)
                nc.vector.tensor_scalar(
                    out=attnT[m][:], in0=attnT[m][:],
                    scalar1=bv_sb[:, m:m + 1], scalar2=None, op0=OP.add)

            def rb_tile(h):
                return nrm_pool.tile([P, TQ], BF16, name=f"rb{h}",
                                     tag=f"rb{h % 4}", bufs=1)
            rb_tiles = {}

            # kq production runs one chunk ahead of head consumption, its
            # halves interleaved between the heads of the previous chunk
            kq_cache = {0: kq_tiles(0)}
            emit_kq_half(0, *kq_cache[0], 0)
            emit_kq_half(0, *kq_cache[0], 1)
            for m in range(KC):
                rb_tiles[m * HPC] = rb_tile(m * HPC)
                rb_tiles[m * HPC + 1] = rb_tile(m * HPC + 1)
                if m + 1 < KC:
                    kq_cache[m + 1] = kq_tiles(m + 1)
                    emit_kq_half(m + 1, *kq_cache[m + 1], 0)
                emit_att_head(m * HPC, *kq_cache[m])
                if m + 1 < KC:
                    emit_kq_half(m + 1, *kq_cache[m + 1], 1)
                emit_att_head(m * HPC + 1, *kq_cache[m])
                emit_att_finalize(m)
                del rb_tiles[m * HPC]
                del rb_tiles[m * HPC + 1]
                del kq_cache[m]

            # q/k/v no longer needed once attention is done
            qkv_scope.close()

            # ================= phase 3: out-proj + residual + LN1 ========
            lnp_pool = top.enter_context(tc.tile_pool(name="lnp", bufs=1))
            g1_bc = lnp_pool.tile([P, C], FP32, name="g1_bc", tag="g1_bc")
            bff2_bc = lnp_pool.tile([P, C], FP32, name="bff2_bc",
                                    tag="bff2_bc")
            g2_bc = lnp_pool.tile([P, C], FP32, name="g2_bc", tag="g2_bc")
            be2_bc = lnp_pool.tile([P, C], FP32, name="be2_bc", tag="be2_bc")
            nc.sync.dma_start(out=g1_bc[:], in_=bcast_view(g1, C))
            nc.sync.dma_start(out=bff2_bc[:], in_=bcast_view(bff2, C))
            nc.sync.dma_start(out=g2_bc[:], in_=bcast_view(g2, C))
            nc.sync.dma_start(out=be2_bc[:], in_=bcast_view(be2, C))
            h_pool = top.enter_context(tc.tile_pool(name="hpool", bufs=1))
            h_sb = [h_pool.tile([P, C], FP32, name=f"h{tq}", tag=f"h{tq}")
                    for tq in range(TQB)]

            with contextlib.ExitStack() as ph3:
                ps3_pool = ph3.enter_context(
                    tc.tile_pool(name="ps3", bufs=3, space="PSUM"))
                pst_pool = ph3.enter_context(
                    tc.tile_pool(name="pst", bufs=2, space="PSUM"))
                hb_pool = ph3.enter_context(tc.tile_pool(name="hb", bufs=2))
                xr_pool = ph3.enter_context(tc.tile_pool(name="xr", bufs=2))
                st_pool = ph3.enter_context(tc.tile_pool(name="st3", bufs=1))

                for tq in range(TQB):
                    xr = xr_pool.tile([P, C], FP32, name="xr", tag="xr",
                                      bufs=2)
                    # residual with b_out pre-added host-side
                    nc.sync.dma_start(out=xr[:],
                                      in_=xres[tq * P:(tq + 1) * P, :])
                    psp = ps3_pool.tile([P, C], FP32, name="psp", tag="psp",
                                        bufs=3)
                    for kc in range(KC):
                        for (no, nsz) in _chunks(C, NMAX):
                            nc.tensor.matmul(
                                psp[:, no:no + nsz],
                                attnT[kc][:, tq * P:(tq + 1) * P],
                                wout_sb[kc][:, no:no + nsz],
                                start=(kc == 0), stop=(kc == KC - 1))
                    hpre = h_sb[tq]
                    nc.vector.tensor_tensor(out=hpre[:], in0=psp[:],
                                            in1=xr[:], op=OP.add)
                    layernorm(hpre, hpre, None, st_pool)
                    # bf16 copy of post-LN h feeds the transposes for FF1
                    hb = hb_pool.tile([P, C], BF16, name="hb", tag="hb",
                                      bufs=2)
                    nc.scalar.copy(out=hb[:], in_=hpre[:])
                    # transpose h -> hT (bf16) via PE
                    for cg in range(0, KC, 4):
                        ncg = min(4, KC - cg)
                        pst = pst_pool.tile([P, NMAX], BF16, name="pst",
                                            tag="pst", bufs=2)
                        for j in range(ncg):
                            nc.tensor.transpose(
                                pst[:, j * P:(j + 1) * P],
                                hb[:, (cg + j) * P:(cg + j + 1) * P],
                                ident[:])
                        for j in range(ncg):
                            nc.scalar.copy(
                                out=hT_sb[cg + j][:, tq * P:(tq + 1) * P],
                                in_=pst[:, j * P:(j + 1) * P])

            # attnT/wout dead now; free the space for FFN weights
            attn_scope.close()
            w1gB_scope = contextlib.ExitStack()
            w1g_poolB = w1gB_scope.enter_context(
                tc.tile_pool(name="w1gB", bufs=1, side="right"))

            # FF1 weight set B + rotations for sets A/B: SP queue (idle
            # now that phase-3 residual loads are queued).
            w1g_sets["B"] = [
                w1g_poolB.tile([P, NG1 * P], BF16, name=f"w1gB_{kc}",
                               tag=f"w1gB{kc}", bufs=1)
                for kc in range(KC)]
            if n_groups > 1:
                for kc in range(KC):
                    nc.sync.dma_start(
                        out=w1g_sets["B"][kc][:],
                        in_=wff1[kc * P:(kc + 1) * P, NG1 * P:2 * NG1 * P])
            # groups 2/3 rotate into the A/B tags (WAR-gated on FF1 reads)
            w1g_rot = {}
            for g in range(2, n_groups):
                si = "AB"[g % 2]
                pool = w1g_poolA if si == "A" else w1g_poolB
                tiles = [pool.tile([P, NG1 * P], BF16, name=f"w1g{g}_{kc}",
                                   tag=f"w1g{si}{kc}", bufs=1)
                         for kc in range(KC)]
                mg = g * NG1
                nmg = min(NG1, MF - mg)
                for kc in range(KC):
                    nc.sync.dma_start(
                        out=tiles[kc][:, :nmg * P],
                        in_=wff1[kc * P:(kc + 1) * P,
                                 mg * P:(mg + nmg) * P])
                w1g_rot[g] = tiles

            # ================= phase 4: FFN (FF1) =================
            gT_pool = top.enter_context(tc.tile_pool(name="gT", bufs=1))
            gT_sb = [gT_pool.tile([P, TQ], BF16, name=f"gT{k}",
                                  tag=f"gT{k}") for k in range(MF)]
            # FF2 weight half A ([F, 0:C/2], 4MB): Pool queue, lands during
            # FF1.  Lives through phase 5.
            CH = C // 2
            w2a_pool = top.enter_context(tc.tile_pool(name="w2a", bufs=1))
            K2 = 4  # k-chunks per w2 tile
            w2a = []
            for k2 in range(0, MF, K2):
                nk = min(K2, MF - k2)
                t2 = w2a_pool.tile([P, K2, CH], BF16, name=f"w2a{k2}",
                                   tag=f"w2a{k2}", bufs=1)
                src_ap = bass.AP(
                    tensor=wff2[:].tensor, offset=k2 * P * C,
                    ap=[[C, P], [P * C, nk], [1, CH]])
                nc.gpsimd.dma_start(out=t2[:, :nk, :], in_=src_ap)
                w2a.append(t2)

            with contextlib.ExitStack() as ph4:
                ps4_pool = ph4.enter_context(
                    tc.tile_pool(name="ps4", bufs=2, space="PSUM"))
                for g in range(n_groups):
                    mg = g * NG1
                    nmg = min(NG1, MF - mg)
                    if g < 2:
                        w1g = w1g_sets["AB"[g]]
                    else:
                        w1g = w1g_rot[g]
                    for mi in range(nmg):
                        m = mg + mi
                        psf = ps4_pool.tile([P, TQ], FP32, name="psf",
                                            tag="psf", bufs=2)
                        for kc in range(KC):
                            for (no, nsz) in _chunks(TQ, NMAX):
                                nc.tensor.matmul(
                                    psf[:, no:no + nsz],
                                    w1g[kc][:, mi * P:(mi + 1) * P],
                                    hT_sb[kc][:, no:no + nsz],
                                    start=(kc == 0), stop=(kc == KC - 1))
                        nc.scalar.activation(out=gT_sb[m][:], in_=psf[:],
                                             func=AF.Gelu,
                                             bias=bff1_sb[:, m:m + 1],
                                             scale=1.0)
            # right-stack pops, LIFO: w1gB, hT, w1gA
            w1gB_scope.close()
            hT_scope.close()
            w1gA_scope.close()

            # ================= phase 5: FF2 (tile-major) + LN2 ===========
            # Both C-halves per token tile back-to-back, then the LN2 tail
            # for that tile runs on DVE/Pool while the PE computes the next
            # tile - no barrier at the end of the phase.
            with contextlib.ExitStack() as ph5:
                w2b_pool = ph5.enter_context(tc.tile_pool(name="w2b",
                                                          bufs=1))
                psy_pool = ph5.enter_context(
                    tc.tile_pool(name="psy", bufs=2, space="PSUM"))
                yo_pool = ph5.enter_context(tc.tile_pool(name="yo", bufs=3))
                st_pool2 = ph5.enter_context(tc.tile_pool(name="st5",
                                                          bufs=2))

                # second C-half weights; resident like w2a (landed during
                # FF1/outproj from the SP queue)
                w2b = []
                for k2 in range(0, MF, K2):
                    nk = min(K2, MF - k2)
                    t2 = w2b_pool.tile([P, K2, CH], BF16, name=f"w2b{k2}",
                                       tag=f"w2b{k2}", bufs=1)
                    src_ap = bass.AP(
                        tensor=wff2[:].tensor, offset=k2 * P * C + CH,
                        ap=[[C, P], [P * C, nk], [1, CH]])
                    nc.sync.dma_start(out=t2[:, :nk, :], in_=src_ap)
                    w2b.append(t2)

                for tq in range(TQB):
                    yo = yo_pool.tile([P, C], FP32, name="yo", tag="yo",
                                      bufs=3)
                    for ch, w2t in ((0, w2a), (1, w2b)):
                        co = ch * CH
                        psy = psy_pool.tile([P, CH], FP32, name="psy",
                                            tag="psy", bufs=2)
                        for k in range(MF):
                            nc.tensor.matmul(
                                psy[:],
                                gT_sb[k][:, tq * P:(tq + 1) * P],
                                w2t[k // K2][:, k % K2, :],
                                start=(k == 0), stop=(k == MF - 1))
                        # yo = h*g1 + bff2' (+be1 merged) + ff2
                        nc.vector.scalar_tensor_tensor(
                            out=yo[:, co:co + CH],
                            in0=h_sb[tq][:, co:co + CH], scalar=0.0,
                            in1=g1_bc[:, co:co + CH],
                            op0=OP.add, op1=OP.mult)
                        nc.gpsimd.tensor_tensor(
                            out=yo[:, co:co + CH],
                            in0=yo[:, co:co + CH],
                            in1=bff2_bc[:, co:co + CH], op=OP.add)
                        nc.vector.tensor_tensor(
                            out=yo[:, co:co + CH],
                            in0=psy[:],
                            in1=yo[:, co:co + CH], op=OP.add)
                    layernorm(yo, yo, g2_bc, st_pool2)
                    nc.gpsimd.tensor_tensor(out=yo[:], in0=yo[:],
                                            in1=be2_bc[:], op=OP.add)
                    nc.sync.dma_start(out=y[tq * P:(tq + 1) * P, :],
                                      in_=yo[:])

    with tile.TileContext(nc) as tc:
        for _rep in range(reps):
            emit_body(tc)

    nc.compile()
    return nc


_NC_CACHE = {}


def _get_nc(T, TQ, C, H, F, n_cores=8, reps=1):
    key = (T, TQ, C, H, F, n_cores, reps)
    if key not in _NC_CACHE:
        _NC_CACHE[key] = build_nc(T, TQ, C, H, F, n_cores, reps=reps)
    return _NC_CACHE[key]


def _bf16(a):
    return np.asarray(a).astype(ml_dtypes.bfloat16)


def prepare(x, W_qkv, b_qkv, W_out, b_out, W_ff1, b_ff1, W_ff2, b_ff2,
            g1, beta1, g2, beta2, reps=1):
    """Build (cached) the program and the per-core input maps."""
    x = np.asarray(x, dtype=np.float32)
    B, T, C = x.shape
    H = 16
    F = W_ff1.shape[1]
    n_cores = 8
    SPB = n_cores // B  # query splits per batch
    TQ = T // SPB

    nc = _get_nc(T, TQ, C, H, F, n_cores, reps=reps)

    # LN1's affine transform is folded into the FF1 weights/bias (exact):
    #   gelu((h*g1+be1) @ W1 + b1) = gelu(h @ (g1[:,None]*W1) + (b1+be1@W1))
    # and the residual branch keeps h*g1 + be1 via g1_bc and be1 merged into
    # the FF2 output bias.
    g1f = np.asarray(g1, np.float64)
    be1f = np.asarray(beta1, np.float64)
    wff1_eff = (g1f[:, None] * np.asarray(W_ff1, np.float64)).astype(
        np.float32)
    bff1_eff = (np.asarray(b_ff1, np.float64)
                + be1f @ np.asarray(W_ff1, np.float64)).astype(np.float32)
    bff2_eff = (np.asarray(b_ff2, np.float64) + be1f).astype(np.float32)
    shared = {
        "wqkv": _bf16(W_qkv), "wout": _bf16(W_out),
        "wff1": _bf16(wff1_eff), "wff2": _bf16(W_ff2),
        "bqkv": np.asarray(b_qkv, np.float32),
        "bff1": bff1_eff,
        "bff2": bff2_eff,
        "g1": np.asarray(g1, np.float32),
        "g2": np.asarray(g2, np.float32), "be2": np.asarray(beta2, np.float32),
    }
    bout_f = np.asarray(b_out, np.float32)
    in_maps = []
    for core in range(n_cores):
        b, s = divmod(core, SPB)
        xT = np.ascontiguousarray(x[b].T)  # [C, T]
        own = xT[:, s * TQ:(s + 1) * TQ]
        rest = [xT[:, j * TQ:(j + 1) * TQ] for j in range(SPB) if j != s]
        xTperm = np.concatenate([own] + rest, axis=1)
        in_maps.append(dict(
            shared,
            xTp=_bf16(xTperm),
            xres=np.ascontiguousarray(
                x[b, s * TQ:(s + 1) * TQ, :] + bout_f[None, :]),
        ))
    return nc, in_maps, (B, T, C, TQ, SPB, n_cores)


def kernel(**inputs):
    nc, in_maps, (B, T, C, TQ, SPB, n_cores) = prepare(**inputs)
    res = run_bass_kernel_spmd(nc, in_maps, list(range(n_cores)))
    out = np.empty((B, T, C), dtype=np.float32)
    for core in range(n_cores):
        b, s = divmod(core, SPB)
        out[b, s * TQ:(s + 1) * TQ, :] = res.results[core]["y"]
    return out


# revision 35
# speedup vs baseline: 1.0344x; 1.0344x over previous
"""Fused transformer block (attention + FFN + 2x LayerNorm) on 8 Trainium2
NeuronCores via Bass/Tile.

Sharding: 8 cores = (batch b in 0..3) x (query-half s in 0..1).  Each core
receives the full x[b] (needed for K/V), computes outputs for its half of the
2048 tokens, fully fused on-chip (no collectives).  Matmuls run in bf16 with
fp32 PSUM accumulation; softmax/LayerNorm/residual math in fp32.

Attention layout trick: scores are computed transposed (S^T = K^T.T @ Q^T per
head, keys on partitions), softmax uses exp without max-subtraction (scores
are O(1) by construction), the normalizer is obtained by appending a ones
column to V (row 64 of the AV product = sum of exp), and the AV product comes
out as O^T [head_dim, tokens] which is exactly the lhsT layout the output
projection wants - so no transposes anywhere in attention.

v1.5 changes vs the original baseline:
 - per-head normalizer: ACT row-copy + Pool partition_broadcast + DVE
   reciprocal on [64, n] (the old [1, TQ] single-partition reciprocal was
   3.3us and serialized heads); AV accumulates per query-half in 1-bank
   psum tiles shared with the kq-production pool.
 - kq production runs one chunk ahead of head consumption (tags rotate %3)
   so the PE never idles across the head-transition normalize chains.
 - weight DMAs spread across queues and issued early: FF1 weights prefetch
   during attention/out-proj; FF2 weights half-prefetch during FF1 and the
   kernel streams each FF2 weight byte exactly once (C-split FF2).
 - b_out folded into the residual input host-side; no y-prefill trick.
"""

import sys

for _p in ("/opt/trn_rl_repo",):
    if _p not in sys.path:
        sys.path.insert(0, _p)

import numpy as np
import ml_dtypes

import concourse.bass as bass
import concourse.mybir as mybir
import concourse.tile as tile
from concourse import bacc
from concourse.bass_utils import run_bass_kernel_spmd
from concourse.masks import make_identity

FP32 = mybir.dt.float32
BF16 = mybir.dt.bfloat16
AF = mybir.ActivationFunctionType
OP = mybir.AluOpType

P = 128
NMAX = 512  # max matmul free dim / psum bank fp32 words
LN_EPS = 1e-5


def _chunks(total, size):
    out = []
    o = 0
    while o < total:
        s = min(size, total - o)
        out.append((o, s))
        o += s
    return out


def build_nc(T, TQ, C, H, F, n_cores=8, reps=1, exp_as_copy=False):
    """Build the SPMD single-core program.  D (head dim) = C // H must be 64.

    reps > 1 emits the whole computation multiple times back-to-back inside
    one NEFF (same inputs/outputs) - used only for wall-clock timing."""
    D = C // H
    assert D == 64 and C % P == 0 and T % P == 0 and TQ % P == 0 and F % P == 0
    KC = C // P     # contraction chunks over C
    TB = T // P     # key-token blocks
    TQB = TQ // P   # query-token blocks
    MF = F // P     # FFN hidden blocks
    HPC = P // D    # heads per 128-chunk (=2)

    nc = bacc.Bacc("TRN2", target_bir_lowering=False, debug=False,
                   num_devices=n_cores)

    # ---- DRAM I/O ----
    xTp = nc.dram_tensor("xTp", [C, T], BF16, kind="ExternalInput")
    xres = nc.dram_tensor("xres", [TQ, C], FP32, kind="ExternalInput")
    wqkv = nc.dram_tensor("wqkv", [C, 3 * C], BF16, kind="ExternalInput")
    wout = nc.dram_tensor("wout", [C, C], BF16, kind="ExternalInput")
    wff1 = nc.dram_tensor("wff1", [C, F], BF16, kind="ExternalInput")
    wff2 = nc.dram_tensor("wff2", [F, C], BF16, kind="ExternalInput")
    bqkv = nc.dram_tensor("bqkv", [3 * C], FP32, kind="ExternalInput")
    bff1 = nc.dram_tensor("bff1", [F], FP32, kind="ExternalInput")
    g1 = nc.dram_tensor("g1", [C], FP32, kind="ExternalInput")
    bff2 = nc.dram_tensor("bff2", [C], FP32, kind="ExternalInput")
    g2 = nc.dram_tensor("g2", [C], FP32, kind="ExternalInput")
    be2 = nc.dram_tensor("be2", [C], FP32, kind="ExternalInput")
    y = nc.dram_tensor("y", [TQ, C], FP32, kind="ExternalOutput")

    def col_view(t, n, off=0):
        # [n*P] dram vector -> [P, n] view: (p, m) = t[off + m*P + p]
        return bass.AP(tensor=t[:].tensor, offset=off, ap=[[1, P], [P, n]])

    def bcast_view(t, n):
        # [n] dram vector broadcast across partitions -> [P, n]
        return bass.AP(tensor=t[:].tensor, offset=0, ap=[[0, P], [1, n]])

    import contextlib

    def emit_body(tc):
        with contextlib.ExitStack() as top:
            params = top.enter_context(tc.tile_pool(name="params", bufs=1))

            bq_sb = params.tile([P, KC], FP32, name="bq_sb", tag="bq_sb")
            bk_sb = params.tile([P, KC], FP32, name="bk_sb", tag="bk_sb")
            bv_sb = params.tile([P, KC], FP32, name="bv_sb", tag="bv_sb")
            bff1_sb = params.tile([P, MF], FP32, name="bff1_sb",
                                  tag="bff1_sb")
            eps_sb = params.tile([P, 1], FP32, name="eps_sb", tag="eps_sb")
            nc.vector.memset(eps_sb[:], LN_EPS)
            ident = params.tile([P, P], BF16, name="ident", tag="ident")
            make_identity(nc, ident[:])
            identf = params.tile([P, P], FP32, name="identf", tag="identf")
            make_identity(nc, identf[:])

            def layernorm(dst, src, g_bc, stats_pool):
                """dst[P, C] (any dtype) = LN(src[P, C] fp32) * g."""
                nsub = (C + NMAX - 1) // NMAX
                stats = stats_pool.tile([P, nsub, 6], FP32, name="ln_stats",
                                        tag="ln_stats", bufs=3)
                for i, (o, sz) in enumerate(_chunks(C, NMAX)):
                    nc.vector.bn_stats(out=stats[:, i, :],
                                       in_=src[:, o:o + sz])
                mv = stats_pool.tile([P, 2], FP32, name="ln_mv", tag="ln_mv",
                                     bufs=3)
                nc.vector.bn_aggr(out=mv[:], in_=stats[:])
                rstd = stats_pool.tile([P, 1], FP32, name="ln_rstd",
                                       tag="ln_rstd", bufs=3)
                nc.scalar.activation(out=rstd[:], in_=mv[:, 1:2],
                                     func=AF.Sqrt, bias=eps_sb[:], scale=1.0)
                nc.vector.reciprocal(out=rstd[:], in_=rstd[:])
                nc.vector.tensor_scalar(out=dst[:], in0=src[:],
                                        scalar1=mv[:, 0:1],
                                        scalar2=rstd[:],
                                        op0=OP.subtract, op1=OP.mult)
                if g_bc is not None:
                    nc.vector.scalar_tensor_tensor(
                        out=dst[:], in0=dst[:], scalar=0.0, in1=g_bc[:],
                        op0=OP.add, op1=OP.mult)

            # Right-side SBUF stack, bottom-up: w1gA (lives to FF1 end),
            # hT (ph3..FF1), attnT+wout (attention..ph3), later w1gB.
            # Stack discipline: each closes before anything below it.
            w1gA_scope = contextlib.ExitStack()
            w1g_poolA = w1gA_scope.enter_context(
                tc.tile_pool(name="w1gA", bufs=1, side="right"))
            hT_scope = contextlib.ExitStack()
            hT_pool = hT_scope.enter_context(
                tc.tile_pool(name="hTp", bufs=1, side="right"))
            hT_sb = [hT_pool.tile([P, TQ], BF16, name=f"hT{c}", tag=f"hT{c}")
                     for c in range(KC)]
            attn_scope = contextlib.ExitStack()
            attn_pool = attn_scope.enter_context(
                tc.tile_pool(name="attn", bufs=1, side="right"))
            attnT = [attn_pool.tile([P, TQ], BF16, name=f"attnT{m}",
                                    tag=f"attnT{m}") for m in range(KC)]
            wout_pool = attn_scope.enter_context(
                tc.tile_pool(name="woutp", bufs=1, side="right"))
            wout_sb = [wout_pool.tile([P, C], BF16, name=f"wout{kc}",
                                      tag=f"wout{kc}") for kc in range(KC)]

            # FF1 weight set A (2 of 4 groups): issued on the Pool queue
            # during attention, after the V-phase weights free.
            NG1 = 8  # f-chunks per w1g group
            n_groups = (MF + NG1 - 1) // NG1

            # ========== phases 1+2: QKV projections + attention ==========
            scale = 1.0 / float(np.sqrt(D))
            qkv_scope = contextlib.ExitStack()
            qkv_pool = qkv_scope.enter_context(
                tc.tile_pool(name="qkv", bufs=1))
            v_pack = [qkv_pool.tile([P, H, D + 1], BF16, name=f"v_pack{tb}",
                                    tag=f"v_pack{tb}") for tb in range(TB)]

            xT_pool = qkv_scope.enter_context(tc.tile_pool(name="xT",
                                                           bufs=1))
            w_pool = qkv_scope.enter_context(
                tc.tile_pool(name="wstream", bufs=1))
            # scores psum: [128, TQ] (2 banks) x2
            pss_pool = qkv_scope.enter_context(
                tc.tile_pool(name="pss", bufs=2, space="PSUM"))
            # shared 1-bank psum rotation: kq production rounds, V-phase
            # rounds, and per-half AV accumulators
            ps_pool = qkv_scope.enter_context(
                tc.tile_pool(name="ps1", bufs=4, space="PSUM"))
            es_pool = qkv_scope.enter_context(tc.tile_pool(name="expS",
                                                           bufs=3))
            nrm_pool = qkv_scope.enter_context(tc.tile_pool(name="nrm",
                                                            bufs=1))

            xT_sb = [xT_pool.tile([P, T], BF16, name=f"xT{kc}",
                                  tag=f"xT{kc}") for kc in range(KC)]
            XH = min(2 * P, T)
            for kc in range(KC):
                nc.sync.dma_start(out=xT_sb[kc][:, :XH],
                                  in_=xTp[kc * P:(kc + 1) * P, :XH])
            for kc in range(KC):
                nc.sync.dma_start(out=xT_sb[kc][:, XH:],
                                  in_=xTp[kc * P:(kc + 1) * P, XH:])

            # --- V (normal layout, packed per head with a ones column) ---
            wv_scope = contextlib.ExitStack()
            wv_pool = wv_scope.enter_context(tc.tile_pool(name="wv", bufs=1))
            wv = [wv_pool.tile([P, C], BF16, name=f"wv{kc}", tag=f"wv{kc}")
                  for kc in range(KC)]
            WH = min(NMAX, C)
            for kc in range(KC):
                nc.scalar.dma_start(
                    out=wv[kc][:, :WH],
                    in_=wqkv[kc * P:(kc + 1) * P, 2 * C:2 * C + WH])
            for kc in range(KC):
                if WH < C:
                    nc.scalar.dma_start(
                        out=wv[kc][:, WH:],
                        in_=wqkv[kc * P:(kc + 1) * P, 2 * C + WH:3 * C])
            nc.scalar.dma_start(out=bq_sb[:], in_=col_view(bqkv, KC, 0))
            nc.scalar.dma_start(out=bk_sb[:], in_=col_view(bqkv, KC, C))
            nc.scalar.dma_start(out=bv_sb[:], in_=col_view(bqkv, KC, 2 * C))
            nc.scalar.dma_start(out=bff1_sb[:], in_=col_view(bff1, MF, 0))
            # K/Q weights + wout + LN broadcasts on the SP queue (idle during
            # attention); K first (needed first).
            wk = [w_pool.tile([P, C], BF16, name=f"wk{kc}", tag=f"wk{kc}")
                  for kc in range(KC)]
            for kc in range(KC):
                nc.sync.dma_start(out=wk[kc][:],
                                  in_=wqkv[kc * P:(kc + 1) * P, C:2 * C])
            wq = [w_pool.tile([P, C], BF16, name=f"wq{kc}", tag=f"wq{kc}")
                  for kc in range(KC)]
            for kc in range(KC):
                nc.scalar.dma_start(out=wq[kc][:],
                                  in_=wqkv[kc * P:(kc + 1) * P, 0:C])
            for kc in range(KC):
                nc.sync.dma_start(out=wout_sb[kc][:],
                                  in_=wout[kc * P:(kc + 1) * P, :])

            for tb in range(TB):
                nc.vector.memset(v_pack[tb][:, :, D:D + 1], 1.0)
                for (no, nsz) in _chunks(C, NMAX):
                    psv = ps_pool.tile([P, NMAX], FP32, name="psv",
                                       tag="ps1", bufs=4)
                    for kc in range(KC):
                        nc.tensor.matmul(
                            psv[:, :nsz],
                            xT_sb[kc][:, tb * P:(tb + 1) * P],
                            wv[kc][:, no:no + nsz],
                            start=(kc == 0), stop=(kc == KC - 1))
                    hview = v_pack[tb][:, no // D:(no + nsz) // D, 0:D]
                    nc.vector.tensor_copy(
                        out=hview,
                        in_=psv[:, :nsz].rearrange("p (h d) -> p h d", d=D))
            wv_scope.close()

            # FF1 weight set A: fresh tiles, issued now on the Pool queue so
            # the transfers land during attention.  Groups 2/3 rotate into
            # the same tags later (WAR-gated on FF1's reads).
            w1g_sets = {}
            for si, setpool in (("A", w1g_poolA),):
                w1g_sets[si] = [
                    setpool.tile([P, NG1 * P], BF16, name=f"w1g{si}_{kc}",
                                 tag=f"w1g{si}{kc}", bufs=1)
                    for kc in range(KC)]
            for kc in range(KC):
                nc.gpsimd.dma_start(
                    out=w1g_sets["A"][kc][:],
                    in_=wff1[kc * P:(kc + 1) * P, 0:NG1 * P])

            kq_pool = qkv_scope.enter_context(tc.tile_pool(name="kq",
                                                           bufs=1))

            def kq_tiles(m):
                kT_m = kq_pool.tile([P, T], BF16, name=f"kT_{m}",
                                    tag=f"kT{m % 2}")
                qT_m = kq_pool.tile([P, TQ], BF16, name=f"qT_{m}",
                                    tag=f"qT{m % 2}")
                return kT_m, qT_m

            def emit_kq_half(m, kT_m, qT_m, half):
                """Produce half of kT_m/qT_m.  Split so the emission can be
                interleaved between attention heads - the PE then always has
                independent matmuls to chew on while a head's normalize
                chain drains."""
                kch = _chunks(T, NMAX)
                qch = _chunks(TQ, NMAX)
                kh = kch[:len(kch) // 2] if half == 0 else kch[len(kch) // 2:]
                qh = qch[:len(qch) // 2] if half == 0 else qch[len(qch) // 2:]
                for (no, nsz) in kh:
                    psk = ps_pool.tile([P, NMAX], FP32, name="psk",
                                       tag="ps1", bufs=4)
                    for kc in range(KC):
                        nc.tensor.matmul(
                            psk[:, :nsz],
                            wk[kc][:, m * P:(m + 1) * P],
                            xT_sb[kc][:, no:no + nsz],
                            start=(kc == 0), stop=(kc == KC - 1))
                    nc.vector.tensor_scalar(
                        out=kT_m[:, no:no + nsz], in0=psk[:, :nsz],
                        scalar1=bk_sb[:, m:m + 1], scalar2=None,
                        op0=OP.add)
                for (no, nsz) in qh:
                    psq = ps_pool.tile([P, NMAX], FP32, name="psq",
                                       tag="ps1", bufs=4)
                    for kc in range(KC):
                        nc.tensor.matmul(
                            psq[:, :nsz],
                            wq[kc][:, m * P:(m + 1) * P],
                            xT_sb[kc][:, no:no + nsz],
                            start=(kc == 0), stop=(kc == KC - 1))
                    nc.vector.tensor_scalar(
                        out=qT_m[:, no:no + nsz], in0=psq[:, :nsz],
                        scalar1=bq_sb[:, m:m + 1], scalar2=None,
                        op0=OP.add)


            def emit_att_head(h, kT_m, qT_m):
                """Scores + exp + AV for one head.  attnT gets the
                UN-normalized O^T; the 1/sum(exp) row is stashed in rrec.
                Normalization happens per-chunk later (emit_att_finalize),
                off the PE critical path - the pso psum slots free after a
                single ACT reciprocal + DVE copy."""
                m, hoff = h // HPC, (h % HPC) * D
                # per query-half AV accumulators (1 psum bank each, shared
                # rotation with kq production)
                pso = [ps_pool.tile([P, NMAX], FP32, name=f"pso{i}",
                                    tag="ps1", bufs=4)
                       for i in range(TQ // NMAX)]
                for ts in range(TB):
                    esr = es_pool.tile([P, TQ], BF16, name="esr",
                                       tag="esr", bufs=3)
                    pss = pss_pool.tile([P, TQ], FP32, name="pss",
                                        tag="pss", bufs=2)
                    for (no, nsz) in _chunks(TQ, NMAX):
                        nc.tensor.matmul(
                            pss[:, no:no + nsz],
                            kT_m[hoff:hoff + D, ts * P:(ts + 1) * P],
                            qT_m[hoff:hoff + D, no:no + nsz],
                            start=True, stop=True)
                    nc.scalar.activation(
                        out=esr[:], in_=pss[:],
                        func=(AF.Copy if exp_as_copy else AF.Exp),
                        scale=scale)
                    for i, (no, nsz) in enumerate(_chunks(TQ, NMAX)):
                        nc.tensor.matmul(
                            pso[i][:D + 1, :nsz],
                            v_pack[ts][:, h, :],
                            esr[:, no:no + nsz],
                            start=(ts == 0), stop=(ts == TB - 1))
                rbh = rb_tiles[h]
                for i, (no, nsz) in enumerate(_chunks(TQ, NMAX)):
                    rraw = nrm_pool.tile([1, NMAX], FP32, name="rraw",
                                         tag="rraw", bufs=2)
                    nc.vector.tensor_copy(out=rraw[:, :nsz],
                                          in_=pso[i][D:D + 1, :nsz])
                    rinv = nrm_pool.tile([1, NMAX], FP32, name="rinv",
                                         tag="rinv", bufs=2)
                    nc.vector.reciprocal_approx_fast(out=rinv[:, :nsz],
                                                     in_=rraw[:, :nsz])
                    rbf = nrm_pool.tile([1, NMAX], BF16, name="rbf",
                                        tag="rbf", bufs=2)
                    nc.scalar.copy(out=rbf[:, :nsz], in_=rinv[:, :nsz])
                    nc.gpsimd.partition_broadcast(
                        rbh[:, no:no + nsz], rbf[:, :nsz], channels=P)
                    nc.vector.tensor_copy(
                        out=attnT[m][hoff:hoff + D, no:no + nsz],
                        in_=pso[i][0:D, :nsz])

            def emit_att_finalize(m):
                """Normalize chunk m of attnT (heads 2m, 2m+1) and add the
                V bias.  Runs on DVE, overlapped with later heads."""
                for hh in range(HPC):
                    hoff = hh * D
                    nc.vector.tensor_tensor(
                        out=attnT[m][hoff:hoff + D, :],
                        in0=attnT[m][hoff:hoff + D, :],
                        in1=rb_tiles[m * HPC + hh][hoff:hoff + D, :],
                        op=OP.mult)
                nc.vector.tensor_scalar(
                    out=attnT[m][:], in0=attnT[m][:],
                    scalar1=bv_sb[:, m:m + 1], scalar2=None, op0=OP.add)

            def rb_tile(h):
                return nrm_pool.tile([P, TQ], BF16, name=f"rb{h}",
                                     tag=f"rb{h % 4}", bufs=1)
            rb_tiles = {}

            # kq production runs one chunk ahead of head consumption, its
            # halves interleaved between the heads of the previous chunk
            kq_cache = {0: kq_tiles(0)}
            emit_kq_half(0, *kq_cache[0], 0)
            emit_kq_half(0, *kq_cache[0], 1)
            for m in range(KC):
                rb_tiles[m * HPC] = rb_tile(m * HPC)
                rb_tiles[m * HPC + 1] = rb_tile(m * HPC + 1)
                if m + 1 < KC:
                    kq_cache[m + 1] = kq_tiles(m + 1)
                    emit_kq_half(m + 1, *kq_cache[m + 1], 0)
                emit_att_head(m * HPC, *kq_cache[m])
                if m + 1 < KC:
                    emit_kq_half(m + 1, *kq_cache[m + 1], 1)
                emit_att_head(m * HPC + 1, *kq_cache[m])
                emit_att_finalize(m)
                del rb_tiles[m * HPC]
                del rb_tiles[m * HPC + 1]
                del kq_cache[m]

            # q/k/v no longer needed once attention is done
            qkv_scope.close()

            # ================= phase 3: out-proj + residual + LN1 ========
            lnp_pool = top.enter_context(tc.tile_pool(name="lnp", bufs=1))
            g1_bc = lnp_pool.tile([P, C], FP32, name="g1_bc", tag="g1_bc")
            bff2_bc = lnp_pool.tile([P, C], FP32, name="bff2_bc",
                                    tag="bff2_bc")
            g2_bc = lnp_pool.tile([P, C], FP32, name="g2_bc", tag="g2_bc")
            be2_bc = lnp_pool.tile([P, C], FP32, name="be2_bc", tag="be2_bc")
            nc.sync.dma_start(out=g1_bc[:], in_=bcast_view(g1, C))
            nc.sync.dma_start(out=bff2_bc[:], in_=bcast_view(bff2, C))
            nc.sync.dma_start(out=g2_bc[:], in_=bcast_view(g2, C))
            nc.sync.dma_start(out=be2_bc[:], in_=bcast_view(be2, C))
            h_pool = top.enter_context(tc.tile_pool(name="hpool", bufs=1))
            h_sb = [h_pool.tile([P, C], FP32, name=f"h{tq}", tag=f"h{tq}")
                    for tq in range(TQB)]

            with contextlib.ExitStack() as ph3:
                ps3_pool = ph3.enter_context(
                    tc.tile_pool(name="ps3", bufs=3, space="PSUM"))
                pst_pool = ph3.enter_context(
                    tc.tile_pool(name="pst", bufs=2, space="PSUM"))
                xr_pool = ph3.enter_context(tc.tile_pool(name="xr", bufs=2))
                st_pool = ph3.enter_context(tc.tile_pool(name="st3", bufs=1))

                for tq in range(TQB):
                    xr = xr_pool.tile([P, C], FP32, name="xr", tag="xr",
                                      bufs=2)
                    # residual with b_out pre-added host-side
                    nc.sync.dma_start(out=xr[:],
                                      in_=xres[tq * P:(tq + 1) * P, :])
                    psp = ps3_pool.tile([P, C], FP32, name="psp", tag="psp",
                                        bufs=3)
                    for kc in range(KC):
                        for (no, nsz) in _chunks(C, NMAX):
                            nc.tensor.matmul(
                                psp[:, no:no + nsz],
                                attnT[kc][:, tq * P:(tq + 1) * P],
                                wout_sb[kc][:, no:no + nsz],
                                start=(kc == 0), stop=(kc == KC - 1))
                    hpre = h_sb[tq]
                    nc.vector.tensor_tensor(out=hpre[:], in0=psp[:],
                                            in1=xr[:], op=OP.add)
                    layernorm(hpre, hpre, None, st_pool)
                    # transpose h -> hT via PE straight from fp32 h
                    # (2 cycles/row, but skips a bf16 staging cast on ACT)
                    for cg in range(0, KC, 4):
                        ncg = min(4, KC - cg)
                        pst = pst_pool.tile([P, NMAX], FP32, name="pst",
                                            tag="pst", bufs=2)
                        for j in range(ncg):
                            nc.tensor.transpose(
                                pst[:, j * P:(j + 1) * P],
                                hpre[:, (cg + j) * P:(cg + j + 1) * P],
                                identf[:])
                        for j in range(ncg):
                            nc.scalar.copy(
                                out=hT_sb[cg + j][:, tq * P:(tq + 1) * P],
                                in_=pst[:, j * P:(j + 1) * P])

            # attnT/wout dead now; free the space for FFN weights
            attn_scope.close()
            w1gB_scope = contextlib.ExitStack()
            w1g_poolB = w1gB_scope.enter_context(
                tc.tile_pool(name="w1gB", bufs=1, side="right"))

            # FF1 weight set B + rotations for sets A/B: SP queue (idle
            # now that phase-3 residual loads are queued).
            w1g_sets["B"] = [
                w1g_poolB.tile([P, NG1 * P], BF16, name=f"w1gB_{kc}",
                               tag=f"w1gB{kc}", bufs=1)
                for kc in range(KC)]
            if n_groups > 1:
                for kc in range(KC):
                    nc.sync.dma_start(
                        out=w1g_sets["B"][kc][:],
                        in_=wff1[kc * P:(kc + 1) * P, NG1 * P:2 * NG1 * P])
            # groups 2/3 rotate into the A/B tags (WAR-gated on FF1 reads)
            w1g_rot = {}
            for g in range(2, n_groups):
                si = "AB"[g % 2]
                pool = w1g_poolA if si == "A" else w1g_poolB
                tiles = [pool.tile([P, NG1 * P], BF16, name=f"w1g{g}_{kc}",
                                   tag=f"w1g{si}{kc}", bufs=1)
                         for kc in range(KC)]
                mg = g * NG1
                nmg = min(NG1, MF - mg)
                for kc in range(KC):
                    nc.sync.dma_start(
                        out=tiles[kc][:, :nmg * P],
                        in_=wff1[kc * P:(kc + 1) * P,
                                 mg * P:(mg + nmg) * P])
                w1g_rot[g] = tiles

            # ================= phase 4: FFN (FF1) =================
            gT_pool = top.enter_context(tc.tile_pool(name="gT", bufs=1))
            gT_sb = [gT_pool.tile([P, TQ], BF16, name=f"gT{k}",
                                  tag=f"gT{k}") for k in range(MF)]
            # FF2 weight half A ([F, 0:C/2], 4MB): Pool queue, lands during
            # FF1.  Lives through phase 5.
            CH = C // 2
            w2a_pool = top.enter_context(tc.tile_pool(name="w2a", bufs=1))
            K2 = 4  # k-chunks per w2 tile
            w2a = []
            for k2 in range(0, MF, K2):
                nk = min(K2, MF - k2)
                t2 = w2a_pool.tile([P, K2, CH], BF16, name=f"w2a{k2}",
                                   tag=f"w2a{k2}", bufs=1)
                src_ap = bass.AP(
                    tensor=wff2[:].tensor, offset=k2 * P * C,
                    ap=[[C, P], [P * C, nk], [1, CH]])
                nc.gpsimd.dma_start(out=t2[:, :nk, :], in_=src_ap)
                w2a.append(t2)

            with contextlib.ExitStack() as ph4:
                ps4_pool = ph4.enter_context(
                    tc.tile_pool(name="ps4", bufs=2, space="PSUM"))
                for g in range(n_groups):
                    mg = g * NG1
                    nmg = min(NG1, MF - mg)
                    if g < 2:
                        w1g = w1g_sets["AB"[g]]
                    else:
                        w1g = w1g_rot[g]
                    for mi in range(nmg):
                        m = mg + mi
                        psf = ps4_pool.tile([P, TQ], FP32, name="psf",
                                            tag="psf", bufs=2)
                        for kc in range(KC):
                            for (no, nsz) in _chunks(TQ, NMAX):
                                nc.tensor.matmul(
                                    psf[:, no:no + nsz],
                                    w1g[kc][:, mi * P:(mi + 1) * P],
                                    hT_sb[kc][:, no:no + nsz],
                                    start=(kc == 0), stop=(kc == KC - 1))
                        nc.scalar.activation(out=gT_sb[m][:], in_=psf[:],
                                             func=AF.Gelu,
                                             bias=bff1_sb[:, m:m + 1],
                                             scale=1.0)
            # right-stack pops, LIFO: w1gB, hT, w1gA
            w1gB_scope.close()
            hT_scope.close()
            w1gA_scope.close()

            # ================= phase 5: FF2 (tile-major) + LN2 ===========
            # Both C-halves per token tile back-to-back, then the LN2 tail
            # for that tile runs on DVE/Pool while the PE computes the next
            # tile - no barrier at the end of the phase.
            with contextlib.ExitStack() as ph5:
                w2b_pool = ph5.enter_context(tc.tile_pool(name="w2b",
                                                          bufs=1))
                psy_pool = ph5.enter_context(
                    tc.tile_pool(name="psy", bufs=3, space="PSUM"))
                yo_pool = ph5.enter_context(tc.tile_pool(name="yo", bufs=4))
                st_pool2 = ph5.enter_context(tc.tile_pool(name="st5",
                                                          bufs=2))

                # second C-half weights; resident like w2a (landed during
                # FF1/outproj from the SP queue)
                w2b = []
                for k2 in range(0, MF, K2):
                    nk = min(K2, MF - k2)
                    t2 = w2b_pool.tile([P, K2, CH], BF16, name=f"w2b{k2}",
                                       tag=f"w2b{k2}", bufs=1)
                    src_ap = bass.AP(
                        tensor=wff2[:].tensor, offset=k2 * P * C + CH,
                        ap=[[C, P], [P * C, nk], [1, CH]])
                    nc.sync.dma_start(out=t2[:, :nk, :], in_=src_ap)
                    w2b.append(t2)

                for tq in range(TQB):
                    yo = yo_pool.tile([P, C], FP32, name="yo", tag="yo",
                                      bufs=4)
                    for ch, w2t in ((0, w2a), (1, w2b)):
                        co = ch * CH
                        psy = psy_pool.tile([P, CH], FP32, name="psy",
                                            tag="psy", bufs=3)
                        for k in range(MF):
                            nc.tensor.matmul(
                                psy[:],
                                gT_sb[k][:, tq * P:(tq + 1) * P],
                                w2t[k // K2][:, k % K2, :],
                                start=(k == 0), stop=(k == MF - 1))
                        # yo = h*g1 + bff2' (+be1 merged) + ff2
                        nc.vector.scalar_tensor_tensor(
                            out=yo[:, co:co + CH],
                            in0=h_sb[tq][:, co:co + CH], scalar=0.0,
                            in1=g1_bc[:, co:co + CH],
                            op0=OP.add, op1=OP.mult)
                        nc.gpsimd.tensor_tensor(
                            out=yo[:, co:co + CH],
                            in0=yo[:, co:co + CH],
                            in1=bff2_bc[:, co:co + CH], op=OP.add)
                        nc.vector.tensor_tensor(
                            out=yo[:, co:co + CH],
                            in0=psy[:],
                            in1=yo[:, co:co + CH], op=OP.add)
                    layernorm(yo, yo, g2_bc, st_pool2)
                    nc.gpsimd.tensor_tensor(out=yo[:], in0=yo[:],
                                            in1=be2_bc[:], op=OP.add)
                    nc.sync.dma_start(out=y[tq * P:(tq + 1) * P, :],
                                      in_=yo[:])

    with tile.TileContext(nc) as tc:
        for _rep in range(reps):
            emit_body(tc)

    nc.compile()
    return nc


_NC_CACHE = {}


def _get_nc(T, TQ, C, H, F, n_cores=8, reps=1):
    key = (T, TQ, C, H, F, n_cores, reps)
    if key not in _NC_CACHE:
        _NC_CACHE[key] = build_nc(T, TQ, C, H, F, n_cores, reps=reps)
    return _NC_CACHE[key]


def _bf16(a):
    return np.asarray(a).astype(ml_dtypes.bfloat16)


def prepare(x, W_qkv, b_qkv, W_out, b_out, W_ff1, b_ff1, W_ff2, b_ff2,
            g1, beta1, g2, beta2, reps=1):
    """Build (cached) the program and the per-core input maps."""
    x = np.asarray(x, dtype=np.float32)
    B, T, C = x.shape
    H = 16
    F = W_ff1.shape[1]
    n_cores = 8
    SPB = n_cores // B  # query splits per batch
    TQ = T // SPB

    nc = _get_nc(T, TQ, C, H, F, n_cores, reps=reps)

    # LN1's affine transform is folded into the FF1 weights/bias (exact):
    #   gelu((h*g1+be1) @ W1 + b1) = gelu(h @ (g1[:,None]*W1) + (b1+be1@W1))
    # and the residual branch keeps h*g1 + be1 via g1_bc and be1 merged into
    # the FF2 output bias.
    g1f = np.asarray(g1, np.float64)
    be1f = np.asarray(beta1, np.float64)
    wff1_eff = (g1f[:, None] * np.asarray(W_ff1, np.float64)).astype(
        np.float32)
    bff1_eff = (np.asarray(b_ff1, np.float64)
                + be1f @ np.asarray(W_ff1, np.float64)).astype(np.float32)
    bff2_eff = (np.asarray(b_ff2, np.float64) + be1f).astype(np.float32)
    shared = {
        "wqkv": _bf16(W_qkv), "wout": _bf16(W_out),
        "wff1": _bf16(wff1_eff), "wff2": _bf16(W_ff2),
        "bqkv": np.asarray(b_qkv, np.float32),
        "bff1": bff1_eff,
        "bff2": bff2_eff,
        "g1": np.asarray(g1, np.float32),
        "g2": np.asarray(g2, np.float32), "be2": np.asarray(beta2, np.float32),
    }
    bout_f = np.asarray(b_out, np.float32)
    in_maps = []
    for core in range(n_cores):
        b, s = divmod(core, SPB)
        xT = np.ascontiguousarray(x[b].T)  # [C, T]
        own = xT[:, s * TQ:(s + 1) * TQ]
        rest = [xT[:, j * TQ:(j + 1) * TQ] for j in range(SPB) if j != s]
        xTperm = np.concatenate([own] + rest, axis=1)
        in_maps.append(dict(
            shared,
            xTp=_bf16(xTperm),
            xres=np.ascontiguousarray(
                x[b, s * TQ:(s + 1) * TQ, :] + bout_f[None, :]),
        ))
    return nc, in_maps, (B, T, C, TQ, SPB, n_cores)


def kernel(**inputs):
    nc, in_maps, (B, T, C, TQ, SPB, n_cores) = prepare(**inputs)
    res = run_bass_kernel_spmd(nc, in_maps, list(range(n_cores)))
    out = np.empty((B, T, C), dtype=np.float32)
    for core in range(n_cores):
        b, s = divmod(core, SPB)
        out[b, s * TQ:(s + 1) * TQ, :] = res.results[core]["y"]
    return out
